# revision 18
# baseline (speedup 1.0000x reference)
"""GCNConv kernel for 8 Trainium2 NeuronCores.

Math (see the reference model):
    A      = dense adjacency from edge_list (duplicates accumulate)
    A_self = A + I
    D[j]   = sum_i A_self[i, j]           (column-sum degrees)
    A_s    = D^-1/2 A_self D^-1/2         (row/col scaling)
    out    = A_s @ (H @ W) + b.T

Sharding: 1D row partition of A_s across the 8 cores (1024 output rows
per core).  The host converts edge_list into per-core transposed
normalized-adjacency blocks A_sT[:, core] in bf16 (exact graph
preprocessing; the values are dinv[i]*dinv[j]*count).  Each core
computes the full X = H @ W on device (replicated, cheap: 1 GFLOP)
and then its 1024-row slice of A_s @ X as a K=8192 blocked matmul
with PSUM accumulation, streaming its 16 MB A_sT block from HBM.
"""

import sys

if "/opt/trn_rl_repo" not in sys.path:
    sys.path.insert(0, "/opt/trn_rl_repo")

import ml_dtypes
import numpy as np

import concourse.tile as tile
from concourse import bacc, mybir
from concourse.bass_utils import run_bass_kernel_spmd

N = 8192
D_IN = 256
D_OUT = 256
N_CORES = 8
ROWS = N // N_CORES  # 1024 output rows per core
P = 128
KT = D_IN // P  # 2 contraction tiles for H @ W
JT = N // P  # 64 contraction tiles for A_s @ X
IT = ROWS // P  # 8 output tiles per core

HTC = 16  # H.T chunks
HCW = N // HTC  # 512 columns per chunk

BF16 = mybir.dt.bfloat16
F32 = mybir.dt.float32


def _emit(tc, out, at, ht, w, brep):
    nc = tc.nc
    # DMA issue costs ~0.6 us of serial sequencer time per dma_start
    # (descriptors then spread across the 16 HW queues at full aggregate
    # bandwidth, FIFO per queue).  So transfers are batched into few
    # large dma_starts, issued on SP in priority order: H.T blocks
    # first (phase 1 is DMA-paced), then the A_sT blocks (phase 2).
    # Leading blocks are small so the first tiles land early.
    HT_SIZES = [1, 1, 2, 4, 8]  # in H.T chunks (4 j-tiles each)
    A_SIZES = [1, 1, 2, 4] + [8] * 7  # in j-tiles
    assert sum(HT_SIZES) == HTC and sum(A_SIZES) == JT
    with (
        tc.tile_pool(name="const", bufs=1) as const,
        tc.tile_pool(name="htb", bufs=1) as htpool,
        tc.tile_pool(name="xsb", bufs=1) as xpool,
        tc.tile_pool(name="ablk", bufs=2) as apool,
        tc.tile_pool(name="osb", bufs=4) as opool,
    ):
        w_sb = const.tile([P, KT, D_OUT], BF16)
        nc.scalar.dma_start(w_sb[:], w[:])
        b_sb = const.tile([P, D_OUT], F32)
        nc.scalar.dma_start(b_sb[:], brep[:])

        ht_blocks = []  # (first_chunk, tile)
        c0 = 0
        for bi, csz in enumerate(HT_SIZES):
            htb = htpool.tile(
                [P, csz, KT, HCW], BF16, name=f"htb{bi}", tag=f"htb{bi}"
            )
            nc.sync.dma_start(
                htb[:],
                ht[c0 : c0 + csz].rearrange("c p kt n -> p c kt n"),
            )
            ht_blocks.append((c0, htb))
            c0 += csz

        def ht_lhsT(jt, kt):
            chunk = jt * P // HCW
            for c0, htb in ht_blocks:
                if c0 <= chunk < c0 + htb.shape[1]:
                    j0 = jt * P % HCW
                    return htb[:, chunk - c0, kt, j0 : j0 + P]
            raise AssertionError

        # Phase 1: X = H @ W for all 8192 rows, kept in SBUF as bf16.
        x_sb = xpool.tile([P, JT, D_OUT], BF16)
        with tc.tile_pool(name="ps1", bufs=2, space="PSUM") as ps1pool:
            for jt in range(JT):
                ps = ps1pool.tile([P, D_OUT], F32)
                for kt in range(KT):
                    nc.tensor.matmul(
                        ps[:],
                        ht_lhsT(jt, kt),
                        w_sb[:, kt, :],
                        start=(kt == 0),
                        stop=(kt == KT - 1),
                    )
                nc.vector.tensor_copy(x_sb[:, jt, :], ps[:])

        # Phase 2: out_block = A_s_block @ X, contraction over all of N.
        # 8 interleaved PSUM accumulation groups, one PSUM bank each
        # (start=True clears has-written bits for the whole bank, so
        # concurrent groups must not share a bank); each A_sT block
        # streams once from HBM and feeds all 8 groups.
        with tc.tile_pool(name="acc", bufs=1, space="PSUM") as accpool:
            accs = [
                accpool.tile([P, D_OUT], F32, name=f"acc{it}", tag=f"acc{it}")
                for it in range(IT)
            ]
            jt = 0
            for asz in A_SIZES:
                a_blk = apool.tile(
                    [P, asz, ROWS], BF16, name=f"ab{asz}_{jt}", tag=f"a{asz}"
                )
                nc.sync.dma_start(
                    a_blk[:],
                    at[jt * P : (jt + asz) * P, :].rearrange(
                        "(a p) i -> p a i", p=P
                    ),
                )
                for aj in range(asz):
                    for it in range(IT):
                        nc.tensor.matmul(
                            accs[it][:],
                            a_blk[:, aj, it * P : (it + 1) * P],
                            x_sb[:, jt + aj, :],
                            start=(jt + aj == 0),
                            stop=(jt + aj == JT - 1),
                        )
                jt += asz
            for it in range(IT):
                o = opool.tile([P, D_OUT], F32)
                nc.vector.tensor_add(o[:], accs[it][:], b_sb[:])
                eng = nc.scalar if it % 2 else nc.sync
                eng.dma_start(out[it * P : (it + 1) * P, :], o[:])


def _build_program():
    nc = bacc.Bacc(
        "TRN2", target_bir_lowering=False, debug=False, num_devices=N_CORES
    )
    at = nc.dram_tensor("at", [N, ROWS], BF16, kind="ExternalInput").ap()
    ht = nc.dram_tensor(
        "ht", [HTC, P, KT, HCW], BF16, kind="ExternalInput"
    ).ap()
    w = nc.dram_tensor("w", [P, KT, D_OUT], BF16, kind="ExternalInput").ap()
    brep = nc.dram_tensor("brep", [P, D_OUT], F32, kind="ExternalInput").ap()
    out = nc.dram_tensor("out", [ROWS, D_OUT], F32, kind="ExternalOutput").ap()
    with tile.TileContext(nc) as tc:
        _emit(tc, out, at, ht, w, brep)
    nc.compile()
    return nc


_PROGRAM = None


def _host_preprocess(H, W, b, edge_list):
    """Graph/format preprocessing: edge_list -> per-core bf16 A_sT blocks."""
    bf16 = ml_dtypes.bfloat16
    el = np.asarray(edge_list)
    rows = el[0].astype(np.int64)
    cols = el[1].astype(np.int64)

    deg = np.bincount(cols, minlength=N).astype(np.float64) + 1.0
    dinv = deg**-0.5

    # Merge duplicate edges and the self loops: AT[j, i] = A_self[i, j].
    diag = np.arange(N, dtype=np.int64)
    key = np.concatenate([cols * N + rows, diag * N + diag])
    uk, cnt = np.unique(key, return_counts=True)
    ju = uk // N
    iu = uk % N
    vals = (cnt.astype(np.float64) * dinv[ju] * dinv[iu]).astype(bf16)

    core_of = iu // ROWS
    at_blocks = []
    for c in range(N_CORES):
        m = core_of == c
        blk = np.zeros((N, ROWS), dtype=bf16)
        blk[ju[m], iu[m] - c * ROWS] = vals[m]
        at_blocks.append(blk)

    # H.T packed as [chunk, partition, kt, col]:
    # ht[c, p, kt, n] = H.T[kt*128 + p, c*HCW + n]
    htT = np.asarray(H, dtype=np.float32).T.astype(bf16)  # [D_IN, N]
    ht = np.ascontiguousarray(
        htT.reshape(KT, P, HTC, HCW).transpose(2, 1, 0, 3)
    )
    # W packed as [partition, kt, d]: wb[p, kt, d] = W[kt*128 + p, d]
    wb = np.ascontiguousarray(
        np.asarray(W, dtype=np.float32)
        .astype(bf16)
        .reshape(KT, P, D_OUT)
        .transpose(1, 0, 2)
    )
    brep = np.broadcast_to(
        np.asarray(b, dtype=np.float32).T, (P, D_OUT)
    ).copy()
    return at_blocks, ht, wb, brep


def kernel(H, W, b, edge_list):
    global _PROGRAM
    at_blocks, ht, wb, brep = _host_preprocess(H, W, b, edge_list)
    if _PROGRAM is None:
        _PROGRAM = _build_program()
    in_maps = [
        {"at": at_blocks[c], "ht": ht, "w": wb, "brep": brep}
        for c in range(N_CORES)
    ]
    res = run_bass_kernel_spmd(_PROGRAM, in_maps, list(range(N_CORES)))
    return np.concatenate(
        [res.results[c]["out"] for c in range(N_CORES)], axis=0
    )


# revision 19
# speedup vs baseline: 1.1557x; 1.1557x over previous
"""GCNConv kernel for 8 Trainium2 NeuronCores.

Math (see the reference model):
    A      = dense adjacency from edge_list (duplicates accumulate)
    A_self = A + I
    D[j]   = sum_i A_self[i, j]           (column-sum degrees)
    A_s    = D^-1/2 A_self D^-1/2         (row/col scaling)
    out    = A_s @ (H @ W) + b.T

Sharding: 1D row partition of A_s across the 8 cores (1024 output rows
per core).  The host converts edge_list into per-core transposed
normalized-adjacency blocks A_sT[:, core] in bf16 (exact graph
preprocessing; the values are dinv[i]*dinv[j]*count).  Each core
computes the full X = H @ W on device (replicated, cheap: 1 GFLOP)
and then its 1024-row slice of A_s @ X as a K=8192 blocked matmul
with PSUM accumulation, streaming its 16 MB A_sT block from HBM.
"""

import sys

if "/opt/trn_rl_repo" not in sys.path:
    sys.path.insert(0, "/opt/trn_rl_repo")

import ml_dtypes
import numpy as np

import concourse.tile as tile
from concourse import bacc, mybir
from concourse.bass_utils import run_bass_kernel_spmd

N = 8192
D_IN = 256
D_OUT = 256
N_CORES = 8
ROWS = N // N_CORES  # 1024 output rows per core
P = 128
KT = D_IN // P  # 2 contraction tiles for H @ W
JT = N // P  # 64 contraction tiles for A_s @ X
IT = ROWS // P  # 8 output tiles per core

HTC = 16  # H.T chunks
HCW = N // HTC  # 512 columns per chunk

BF16 = mybir.dt.bfloat16
F32 = mybir.dt.float32


def _emit(tc, out, at, ht, w, brep):
    nc = tc.nc
    # DMA issue costs ~0.6 us of serial sequencer time per dma_start
    # (descriptors then spread across the 16 HW queues at full aggregate
    # bandwidth, FIFO per queue).  So transfers are batched into few
    # large dma_starts, issued on SP in priority order: H.T blocks
    # first (phase 1 is DMA-paced), then the A_sT blocks (phase 2).
    # Leading blocks are small so the first tiles land early.
    HT_SIZES = [1, 1, 2, 4, 8]  # in H.T chunks (4 j-tiles each)
    A_SIZES = [1, 1, 2, 4] + [8] * 7  # in j-tiles
    assert sum(HT_SIZES) == HTC and sum(A_SIZES) == JT
    with (
        tc.tile_pool(name="const", bufs=1) as const,
        tc.tile_pool(name="htb", bufs=1) as htpool,
        tc.tile_pool(name="xsb", bufs=1) as xpool,
        tc.tile_pool(name="ablk", bufs=3) as apool,
        tc.tile_pool(name="osb", bufs=4) as opool,
    ):
        w_sb = const.tile([P, KT, D_OUT], BF16)
        nc.scalar.dma_start(w_sb[:], w[:])
        b_sb = const.tile([P, D_OUT], F32)
        nc.scalar.dma_start(b_sb[:], brep[:])

        ht_blocks = []  # (first_chunk, tile)
        c0 = 0
        for bi, csz in enumerate(HT_SIZES):
            htb = htpool.tile(
                [P, csz, KT, HCW], BF16, name=f"htb{bi}", tag=f"htb{bi}"
            )
            nc.sync.dma_start(
                htb[:],
                ht[c0 : c0 + csz].rearrange("c p kt n -> p c kt n"),
            )
            ht_blocks.append((c0, htb))
            c0 += csz

        def ht_lhsT(jt, kt):
            chunk = jt * P // HCW
            for c0, htb in ht_blocks:
                if c0 <= chunk < c0 + htb.shape[1]:
                    j0 = jt * P % HCW
                    return htb[:, chunk - c0, kt, j0 : j0 + P]
            raise AssertionError

        # Phase 1: X = H @ W for all 8192 rows, kept in SBUF as bf16.
        x_sb = xpool.tile([P, JT, D_OUT], BF16)
        with tc.tile_pool(name="ps1", bufs=6, space="PSUM") as ps1pool:
            for jt in range(JT):
                ps = ps1pool.tile([P, D_OUT], F32)
                for kt in range(KT):
                    nc.tensor.matmul(
                        ps[:],
                        ht_lhsT(jt, kt),
                        w_sb[:, kt, :],
                        start=(kt == 0),
                        stop=(kt == KT - 1),
                    )
                nc.vector.tensor_copy(x_sb[:, jt, :], ps[:])

        # Phase 2: out_block = A_s_block @ X, contraction over all of N.
        # 8 interleaved PSUM accumulation groups, one PSUM bank each
        # (start=True clears has-written bits for the whole bank, so
        # concurrent groups must not share a bank); each A_sT block
        # streams once from HBM and feeds all 8 groups.
        with tc.tile_pool(name="acc", bufs=1, space="PSUM") as accpool:
            accs = [
                accpool.tile([P, D_OUT], F32, name=f"acc{it}", tag=f"acc{it}")
                for it in range(IT)
            ]
            jt = 0
            for asz in A_SIZES:
                a_blk = apool.tile(
                    [P, asz, ROWS], BF16, name=f"ab{asz}_{jt}", tag=f"a{asz}"
                )
                nc.sync.dma_start(
                    a_blk[:],
                    at[jt * P : (jt + asz) * P, :].rearrange(
                        "(a p) i -> p a i", p=P
                    ),
                )
                for aj in range(asz):
                    for it in range(IT):
                        nc.tensor.matmul(
                            accs[it][:],
                            a_blk[:, aj, it * P : (it + 1) * P],
                            x_sb[:, jt + aj, :],
                            start=(jt + aj == 0),
                            stop=(jt + aj == JT - 1),
                        )
                jt += asz
            for it in range(IT):
                o = opool.tile([P, D_OUT], F32)
                nc.vector.tensor_add(o[:], accs[it][:], b_sb[:])
                eng = nc.scalar if it % 2 else nc.sync
                eng.dma_start(out[it * P : (it + 1) * P, :], o[:])


def _build_program():
    nc = bacc.Bacc(
        "TRN2", target_bir_lowering=False, debug=False, num_devices=N_CORES
    )
    at = nc.dram_tensor("at", [N, ROWS], BF16, kind="ExternalInput").ap()
    ht = nc.dram_tensor(
        "ht", [HTC, P, KT, HCW], BF16, kind="ExternalInput"
    ).ap()
    w = nc.dram_tensor("w", [P, KT, D_OUT], BF16, kind="ExternalInput").ap()
    brep = nc.dram_tensor("brep", [P, D_OUT], F32, kind="ExternalInput").ap()
    out = nc.dram_tensor("out", [ROWS, D_OUT], F32, kind="ExternalOutput").ap()
    with tile.TileContext(nc) as tc:
        _emit(tc, out, at, ht, w, brep)
    nc.compile()
    return nc


_PROGRAM = None


def _host_preprocess(H, W, b, edge_list):
    """Graph/format preprocessing: edge_list -> per-core bf16 A_sT blocks."""
    bf16 = ml_dtypes.bfloat16
    el = np.asarray(edge_list)
    rows = el[0].astype(np.int64)
    cols = el[1].astype(np.int64)

    deg = np.bincount(cols, minlength=N).astype(np.float64) + 1.0
    dinv = deg**-0.5

    # Merge duplicate edges and the self loops: AT[j, i] = A_self[i, j].
    diag = np.arange(N, dtype=np.int64)
    key = np.concatenate([cols * N + rows, diag * N + diag])
    uk, cnt = np.unique(key, return_counts=True)
    ju = uk // N
    iu = uk % N
    vals = (cnt.astype(np.float64) * dinv[ju] * dinv[iu]).astype(bf16)

    core_of = iu // ROWS
    at_blocks = []
    for c in range(N_CORES):
        m = core_of == c
        blk = np.zeros((N, ROWS), dtype=bf16)
        blk[ju[m], iu[m] - c * ROWS] = vals[m]
        at_blocks.append(blk)

    # H.T packed as [chunk, partition, kt, col]:
    # ht[c, p, kt, n] = H.T[kt*128 + p, c*HCW + n]
    htT = np.asarray(H, dtype=np.float32).T.astype(bf16)  # [D_IN, N]
    ht = np.ascontiguousarray(
        htT.reshape(KT, P, HTC, HCW).transpose(2, 1, 0, 3)
    )
    # W packed as [partition, kt, d]: wb[p, kt, d] = W[kt*128 + p, d]
    wb = np.ascontiguousarray(
        np.asarray(W, dtype=np.float32)
        .astype(bf16)
        .reshape(KT, P, D_OUT)
        .transpose(1, 0, 2)
    )
    brep = np.broadcast_to(
        np.asarray(b, dtype=np.float32).T, (P, D_OUT)
    ).copy()
    return at_blocks, ht, wb, brep


def kernel(H, W, b, edge_list):
    global _PROGRAM
    at_blocks, ht, wb, brep = _host_preprocess(H, W, b, edge_list)
    if _PROGRAM is None:
        _PROGRAM = _build_program()
    in_maps = [
        {"at": at_blocks[c], "ht": ht, "w": wb, "brep": brep}
        for c in range(N_CORES)
    ]
    res = run_bass_kernel_spmd(_PROGRAM, in_maps, list(range(N_CORES)))
    return np.concatenate(
        [res.results[c]["out"] for c in range(N_CORES)], axis=0
    )


# revision 20
# speedup vs baseline: 1.2557x; 1.0865x over previous
"""GCNConv kernel for 8 Trainium2 NeuronCores.

Math (see the reference model):
    A      = dense adjacency from edge_list (duplicates accumulate)
    A_self = A + I
    D[j]   = sum_i A_self[i, j]           (column-sum degrees)
    A_s    = D^-1/2 A_self D^-1/2         (row/col scaling)
    out    = A_s @ (H @ W) + b.T

Sharding: 1D row partition of A_s across the 8 cores (1024 output rows
per core).  The host converts edge_list into per-core transposed
normalized-adjacency blocks A_sT[:, core] in bf16 (exact graph
preprocessing; the values are dinv[i]*dinv[j]*count).  Each core
computes the full X = H @ W on device (replicated, cheap: 1 GFLOP)
and then its 1024-row slice of A_s @ X as a K=8192 blocked matmul
with PSUM accumulation, streaming its 16 MB A_sT block from HBM.
"""

import sys

if "/opt/trn_rl_repo" not in sys.path:
    sys.path.insert(0, "/opt/trn_rl_repo")

import ml_dtypes
import numpy as np

import concourse.tile as tile
from concourse import bacc, mybir
from concourse.bass_utils import run_bass_kernel_spmd

N = 8192
D_IN = 256
D_OUT = 256
N_CORES = 8
ROWS = N // N_CORES  # 1024 output rows per core
P = 128
KT = D_IN // P  # 2 contraction tiles for H @ W
JT = N // P  # 64 contraction tiles for A_s @ X
IT = ROWS // P  # 8 output tiles per core

HTC = 16  # H.T chunks
HCW = N // HTC  # 512 columns per chunk

BF16 = mybir.dt.bfloat16
F32 = mybir.dt.float32


def _emit(tc, out, at, ht, w, brep):
    nc = tc.nc
    # DMA issue costs ~0.6 us of serial sequencer time per dma_start
    # (descriptors then spread across the 16 HW queues at full aggregate
    # bandwidth, FIFO per queue).  So transfers are batched into few
    # large dma_starts, issued on SP in priority order: H.T blocks
    # first (phase 1 is DMA-paced), then the A_sT blocks (phase 2).
    # Leading blocks are small so the first tiles land early.
    HT_SIZES = [1, 1, 2, 4, 8]  # in H.T chunks (4 j-tiles each)
    A_SIZES = [1, 1, 2, 4] + [8] * 7  # in j-tiles
    assert sum(HT_SIZES) == HTC and sum(A_SIZES) == JT
    with (
        tc.tile_pool(name="const", bufs=1) as const,
        tc.tile_pool(name="htb", bufs=1) as htpool,
        tc.tile_pool(name="xsb", bufs=1) as xpool,
        tc.tile_pool(name="ablk", bufs=3) as apool,
        tc.tile_pool(name="osb", bufs=4) as opool,
    ):
        w_sb = const.tile([P, KT, D_OUT], BF16)
        nc.scalar.dma_start(w_sb[:], w[:])

        ht_blocks = []  # (first_chunk, tile)
        c0 = 0
        for bi, csz in enumerate(HT_SIZES):
            htb = htpool.tile(
                [P, csz, KT, HCW], BF16, name=f"htb{bi}", tag=f"htb{bi}"
            )
            nc.sync.dma_start(
                htb[:],
                ht[c0 : c0 + csz].rearrange("c p kt n -> p c kt n"),
            )
            ht_blocks.append((c0, htb))
            c0 += csz

        def ht_lhsT(jt, kt):
            chunk = jt * P // HCW
            for c0, htb in ht_blocks:
                if c0 <= chunk < c0 + htb.shape[1]:
                    j0 = jt * P % HCW
                    return htb[:, chunk - c0, kt, j0 : j0 + P]
            raise AssertionError

        b_sb = const.tile([P, D_OUT], F32)
        nc.scalar.dma_start(b_sb[:], brep[:])

        # Phase 1: X = H @ W for all 8192 rows, kept in SBUF as bf16.
        # The PSUM -> SBUF evacuation alternates between DVE and ACT so
        # the copies (417/560 ns each) keep up with the PE matmul pair
        # rate (~230 ns per j-tile).
        x_sb = xpool.tile([P, JT, D_OUT], BF16)
        with tc.tile_pool(name="ps1", bufs=6, space="PSUM") as ps1pool:
            for jt in range(JT):
                ps = ps1pool.tile([P, D_OUT], F32)
                for kt in range(KT):
                    nc.tensor.matmul(
                        ps[:],
                        ht_lhsT(jt, kt),
                        w_sb[:, kt, :],
                        start=(kt == 0),
                        stop=(kt == KT - 1),
                    )
                if jt % 2 == 0:
                    nc.vector.tensor_copy(x_sb[:, jt, :], ps[:])
                else:
                    nc.scalar.copy(x_sb[:, jt, :], ps[:])

        # Phase 2: out_block = A_s_block @ X, contraction over all of N.
        # 8 interleaved PSUM accumulation groups, one PSUM bank each
        # (start=True clears has-written bits for the whole bank, so
        # concurrent groups must not share a bank); each A_sT block
        # streams once from HBM and feeds all 8 groups.
        with tc.tile_pool(name="acc", bufs=1, space="PSUM") as accpool:
            accs = [
                accpool.tile([P, D_OUT], F32, name=f"acc{it}", tag=f"acc{it}")
                for it in range(IT)
            ]
            jt = 0
            for bi, asz in enumerate(A_SIZES):
                last_blk = bi == len(A_SIZES) - 1
                a_blk = apool.tile(
                    [P, asz, ROWS], BF16, name=f"ab{asz}_{jt}", tag=f"a{asz}"
                )
                nc.sync.dma_start(
                    a_blk[:],
                    at[jt * P : (jt + asz) * P, :].rearrange(
                        "(a p) i -> p a i", p=P
                    ),
                )
                if not last_blk:
                    for aj in range(asz):
                        for it in range(IT):
                            nc.tensor.matmul(
                                accs[it][:],
                                a_blk[:, aj, it * P : (it + 1) * P],
                                x_sb[:, jt + aj, :],
                                start=(jt + aj == 0),
                                stop=False,
                            )
                else:
                    # it-major on the final block: each accumulation
                    # group closes early, so its bias-add and store
                    # overlap the remaining groups' matmuls.
                    for it in range(IT):
                        for aj in range(asz):
                            nc.tensor.matmul(
                                accs[it][:],
                                a_blk[:, aj, it * P : (it + 1) * P],
                                x_sb[:, jt + aj, :],
                                start=False,
                                stop=(aj == asz - 1),
                            )
                        o = opool.tile([P, D_OUT], F32, name=f"o{it}")
                        nc.vector.tensor_add(o[:], accs[it][:], b_sb[:])
                        eng = nc.scalar if it % 2 else nc.sync
                        eng.dma_start(out[it * P : (it + 1) * P, :], o[:])
                jt += asz


def _build_program():
    nc = bacc.Bacc(
        "TRN2", target_bir_lowering=False, debug=False, num_devices=N_CORES
    )
    at = nc.dram_tensor("at", [N, ROWS], BF16, kind="ExternalInput").ap()
    ht = nc.dram_tensor(
        "ht", [HTC, P, KT, HCW], BF16, kind="ExternalInput"
    ).ap()
    w = nc.dram_tensor("w", [P, KT, D_OUT], BF16, kind="ExternalInput").ap()
    brep = nc.dram_tensor("brep", [P, D_OUT], F32, kind="ExternalInput").ap()
    out = nc.dram_tensor("out", [ROWS, D_OUT], F32, kind="ExternalOutput").ap()
    with tile.TileContext(nc) as tc:
        _emit(tc, out, at, ht, w, brep)
    nc.compile()
    return nc


_PROGRAM = None


def _host_preprocess(H, W, b, edge_list):
    """Graph/format preprocessing: edge_list -> per-core bf16 A_sT blocks."""
    bf16 = ml_dtypes.bfloat16
    el = np.asarray(edge_list)
    rows = el[0].astype(np.int64)
    cols = el[1].astype(np.int64)

    deg = np.bincount(cols, minlength=N).astype(np.float64) + 1.0
    dinv = deg**-0.5

    # Merge duplicate edges and the self loops: AT[j, i] = A_self[i, j].
    diag = np.arange(N, dtype=np.int64)
    key = np.concatenate([cols * N + rows, diag * N + diag])
    uk, cnt = np.unique(key, return_counts=True)
    ju = uk // N
    iu = uk % N
    vals = (cnt.astype(np.float64) * dinv[ju] * dinv[iu]).astype(bf16)

    core_of = iu // ROWS
    at_blocks = []
    for c in range(N_CORES):
        m = core_of == c
        blk = np.zeros((N, ROWS), dtype=bf16)
        blk[ju[m], iu[m] - c * ROWS] = vals[m]
        at_blocks.append(blk)

    # H.T packed as [chunk, partition, kt, col]:
    # ht[c, p, kt, n] = H.T[kt*128 + p, c*HCW + n]
    htT = np.asarray(H, dtype=np.float32).T.astype(bf16)  # [D_IN, N]
    ht = np.ascontiguousarray(
        htT.reshape(KT, P, HTC, HCW).transpose(2, 1, 0, 3)
    )
    # W packed as [partition, kt, d]: wb[p, kt, d] = W[kt*128 + p, d]
    wb = np.ascontiguousarray(
        np.asarray(W, dtype=np.float32)
        .astype(bf16)
        .reshape(KT, P, D_OUT)
        .transpose(1, 0, 2)
    )
    brep = np.broadcast_to(
        np.asarray(b, dtype=np.float32).T, (P, D_OUT)
    ).copy()
    return at_blocks, ht, wb, brep


def kernel(H, W, b, edge_list):
    global _PROGRAM
    at_blocks, ht, wb, brep = _host_preprocess(H, W, b, edge_list)
    if _PROGRAM is None:
        _PROGRAM = _build_program()
    in_maps = [
        {"at": at_blocks[c], "ht": ht, "w": wb, "brep": brep}
        for c in range(N_CORES)
    ]
    res = run_bass_kernel_spmd(_PROGRAM, in_maps, list(range(N_CORES)))
    return np.concatenate(
        [res.results[c]["out"] for c in range(N_CORES)], axis=0
    )
